# revision 12
# baseline (speedup 1.0000x reference)
"""DiceLoss kernel v2: fp8 x + DoubleRow matmuls + host-side diag extraction.

Per core (1/8 of the X axis): x is shipped as fp8e4 in a [33, 128, 2, 864]
layout (voxel v = q*256 + t*128 + p), labels as bf16 [128, 1728]
(flat col j = t*864 + q). Per class c:
  sumsq  = diag of DoubleRow x^T x (fp8, 0.5 cyc/row on PE)
  intersect = diag of mask^T x; for DR_CLASSES the mask is fp8 and the
    matmul runs DoubleRow; otherwise the mask is bf16 (DVE 4x build) and
    the matmul streams x fp8 plain (1 cyc/row).
Each class accumulates its diagonals into a 256B/512B PSUM slot
(bank-strided, reuse gap 8 classes); DoubleRow outputs must sit at PSUM
partition 0, so DR classes use a [64,128] slot (int|sq) and mixed classes
a [128,64] slot (sq at partitions 0-63, intersect at 64-127). ACT copies
each finished slot to a bf16 stats tile; the diagonals are extracted on
the host, which also does bincount(labels) and the final dice ratio,
like the previous version.
"""
import numpy as np
import ml_dtypes
import concourse.bacc as bacc
import concourse.mybir as mybir
import concourse.tile as tile
from concourse.bass_utils import run_bass_kernel_spmd

N_CORES = 8
B, C, X, Y, Z = 2, 33, 96, 96, 96
XS = X // N_CORES
VOX = B * XS * Y * Z          # 221184
P = 128
QC = VOX // 256               # 864 q-cols
FB = 2 * QC                   # 1728 flat cols
W = 64                        # matmul window (psum diag width)
SMOOTH = 1e-5

# classes whose intersect runs DoubleRow with an fp8 mask (tunable split:
# fp8 masks cost ~2x on DVE but their matmuls cost ~4x less on PE)
DR_CLASSES = frozenset({2, 5, 8, 11, 14, 17, 20, 23, 26, 28, 30, 31, 32})

_cached = {}


def _windows(total, w):
    out = []
    o = 0
    while o < total:
        out.append((o, min(w, total - o)))
        o += w
    return out


def _build(dr_classes=DR_CLASSES):
    n_dr = len(dr_classes)
    n_mx = C - n_dr
    nc = bacc.Bacc("TRN2", target_bir_lowering=False, debug=False,
                   num_devices=N_CORES)
    f8 = mybir.dt.float8e4
    bf = mybir.dt.bfloat16
    f32 = mybir.dt.float32
    x_in = nc.dram_tensor("x", [C, P, 2, QC], f8, kind="ExternalInput")
    lab_in = nc.dram_tensor("lab", [P, FB], bf, kind="ExternalInput")
    st_mx_out = nc.dram_tensor("st_mx", [P, W * n_mx], bf,
                               kind="ExternalOutput")
    st_dr_out = nc.dram_tensor("st_dr", [W, 2 * W * n_dr], bf,
                               kind="ExternalOutput")

    # psum slot assignment: bank = order-index % 8, cursor per bank
    cursors = [0] * 8
    slot_off = {}
    for i in range(C):
        bank = i % 8
        size = 512 if i in dr_classes else 256
        byte_off = bank * 2048 + cursors[bank]
        assert cursors[bank] + size <= 2048
        cursors[bank] += size
        slot_off[i] = byte_off // 4       # f32 column offset

    with tile.TileContext(nc) as tc:
        with (
            tc.tile_pool(name="xp", bufs=1) as xp,
            tc.tile_pool(name="labp", bufs=1) as labp,
            tc.tile_pool(name="maskp8", bufs=3) as maskp8,
            tc.tile_pool(name="maskpb", bufs=4) as maskpb,
            tc.tile_pool(name="statp", bufs=1) as statp,
            tc.tile_pool(name="psum", bufs=1, space="PSUM") as psp,
        ):
            lab = labp.tile([P, FB], bf)
            xt = xp.tile([P, C, 2, QC], f8)
            # x DMA groups: one class, then labels, then the rest
            nc.sync.dma_start(xt[:, 0:1], x_in[0:1])
            nc.sync.dma_start(lab[:], lab_in[:, :])
            for c0, c1 in [(1, 2), (2, 3), (3, 5), (5, 8), (8, 13), (13, 18),
                           (18, 23), (23, 28), (28, 33)]:
                nc.sync.dma_start(xt[:, c0:c1], x_in[c0:c1])

            ps = psp.tile([P, 4096], f32)
            st_mx = statp.tile([P, W * n_mx], bf)
            st_dr = statp.tile([W, 2 * W * n_dr], bf)

            wl = _windows(QC, W)
            mi = di = 0
            mx_shipped = dr_shipped = 0
            for c in range(C):
                sc = slot_off[c]
                xc = xt[:, c]
                if c in dr_classes:
                    # sumsq (DR) first: needs no mask, keeps PE fed early.
                    # start=True on w0 marks the whole 2KB zero region for
                    # partitions 0-63, so the intersect below runs start=False.
                    for j, (o, m) in enumerate(wl):
                        nc.tensor.matmul(
                            ps[0:m, sc + W:sc + W + m],
                            xc[:, :, o:o + m],
                            xc[:, :, o:o + m],
                            start=(j == 0), stop=False,
                            perf_mode=mybir.MatmulPerfMode.DoubleRow,
                            skip_group_check=True)
                    mask8 = maskp8.tile([P, 2, QC], f8)
                    nc.vector.tensor_scalar(mask8[:], lab[:], float(c), None,
                                            mybir.AluOpType.is_equal)
                    # intersect (DR) into cols sc..sc+64, partitions 0-63
                    for j, (o, m) in enumerate(wl):
                        nc.tensor.matmul(
                            ps[0:m, sc:sc + m],
                            mask8[:, :, o:o + m],
                            xc[:, :, o:o + m],
                            start=False, stop=(j == len(wl) - 1),
                            perf_mode=mybir.MatmulPerfMode.DoubleRow,
                            skip_group_check=True)
                    nc.scalar.copy(st_dr[:, di * 2 * W:(di + 1) * 2 * W],
                                   ps[0:W, sc:sc + 2 * W])
                    di += 1
                else:
                    # sumsq (DR) at partitions 0-63
                    for j, (o, m) in enumerate(wl):
                        nc.tensor.matmul(
                            ps[0:m, sc:sc + m],
                            xc[:, :, o:o + m],
                            xc[:, :, o:o + m],
                            start=(j == 0), stop=(j == len(wl) - 1),
                            perf_mode=mybir.MatmulPerfMode.DoubleRow,
                            skip_group_check=True)
                    maskb = maskpb.tile([P, 2, QC], bf)
                    nc.vector.tensor_scalar(maskb[:], lab[:], float(c), None,
                                            mybir.AluOpType.is_equal)
                    # intersect (plain, bf16 mask stationary) at partitions
                    # 64-127 (own zero-region marking: partition-range aware)
                    k = 0
                    nwin = 2 * len(wl)
                    for t in range(2):
                        for (o, m) in wl:
                            nc.tensor.matmul(
                                ps[W:W + m, sc:sc + m],
                                maskb[:, t, o:o + m],
                                xc[:, t, o:o + m],
                                start=(k == 0), stop=(k == nwin - 1),
                                skip_group_check=True)
                            k += 1
                    nc.scalar.copy(st_mx[:, mi * W:(mi + 1) * W],
                                   ps[:, sc:sc + W])
                    mi += 1
                # stream finished stats out so the tail DMA stays small
                if c in (11, 19, 25, 28, 30, 31):
                    if mi > mx_shipped:
                        nc.sync.dma_start(
                            st_mx_out[:, mx_shipped * W:mi * W],
                            st_mx[:, mx_shipped * W:mi * W])
                        mx_shipped = mi
                    if di > dr_shipped:
                        nc.sync.dma_start(
                            st_dr_out[:, dr_shipped * 2 * W:di * 2 * W],
                            st_dr[:, dr_shipped * 2 * W:di * 2 * W])
                        dr_shipped = di
            if mi > mx_shipped:
                nc.sync.dma_start(st_mx_out[:, mx_shipped * W:mi * W],
                                  st_mx[:, mx_shipped * W:mi * W])
            if di > dr_shipped:
                nc.sync.dma_start(st_dr_out[:, dr_shipped * 2 * W:di * 2 * W],
                                  st_dr[:, dr_shipped * 2 * W:di * 2 * W])
    nc.compile()
    return nc


def _get_nc():
    if "nc" not in _cached:
        _cached["nc"] = _build()
    return _cached["nc"]


def kernel(outputs, label):
    nc = _get_nc()
    outputs = np.asarray(outputs)
    lab_np = np.asarray(label)
    f8 = ml_dtypes.bfloat16  # placeholder; real fp8 below
    fp8 = ml_dtypes.float8_e4m3
    bf16 = ml_dtypes.bfloat16
    in_maps = []
    for k in range(N_CORES):
        xs = outputs[:, :, k * XS:(k + 1) * XS]          # [B, C, XS, Y, Z]
        xs = np.ascontiguousarray(xs.transpose(1, 0, 2, 3, 4)).reshape(C, VOX)
        # v = q*256 + t*128 + p  ->  [C, 864, 2, 128] -> [C, 128, 2, 864]
        xd = np.ascontiguousarray(
            xs.reshape(C, QC, 2, P).transpose(0, 3, 2, 1)).astype(fp8)
        ls = lab_np[:, k * XS:(k + 1) * XS].reshape(VOX)
        ld = np.ascontiguousarray(
            ls.reshape(QC, 2, P).transpose(2, 1, 0)).reshape(P, FB)
        in_maps.append({"x": xd, "lab": ld.astype(bf16)})
    res = run_bass_kernel_spmd(nc, in_maps, core_ids=list(range(N_CORES)))

    dr_sorted = sorted(DR_CLASSES)
    mx_sorted = [c for c in range(C) if c not in DR_CLASSES]
    intersect = np.zeros(C, np.float64)
    sumsq = np.zeros(C, np.float64)
    idx = np.arange(W)
    for r in res.results:
        smx = r["st_mx"].astype(np.float64)   # [128, 64*n_mx]
        sdr = r["st_dr"].astype(np.float64)   # [64, 128*n_dr]
        for mi, c in enumerate(mx_sorted):
            blk = smx[:, mi * W:(mi + 1) * W]
            sumsq[c] += blk[idx, idx].sum()
            intersect[c] += blk[W + idx, idx].sum()
        for di, c in enumerate(dr_sorted):
            blk = sdr[:, di * 2 * W:(di + 1) * 2 * W]
            intersect[c] += blk[idx, idx].sum()
            sumsq[c] += blk[idx, W + idx].sum()
    labels_sum = np.bincount(
        lab_np.reshape(-1).astype(np.int64), minlength=C).astype(np.float64)
    dice = (2.0 * intersect + SMOOTH) / (sumsq + labels_sum + SMOOTH)
    return np.float32(np.mean(1.0 - dice))


# revision 14
# speedup vs baseline: 1.0120x; 1.0120x over previous
"""DiceLoss kernel v2: fp8 x + DoubleRow matmuls + host-side diag extraction.

Per core (1/8 of the X axis): x is shipped as fp8e4 in a [33, 128, 2, 864]
layout (voxel v = q*256 + t*128 + p), labels as bf16 [128, 1728]
(flat col j = t*864 + q). Per class c:
  sumsq  = diag of DoubleRow x^T x (fp8, 0.5 cyc/row on PE)
  intersect = diag of mask^T x; for DR_CLASSES the mask is fp8 and the
    matmul runs DoubleRow; otherwise the mask is bf16 (DVE 4x build) and
    the matmul streams x fp8 plain (1 cyc/row).
Each class accumulates its diagonals into a 256B/512B PSUM slot
(bank-strided, reuse gap 8 classes); DoubleRow outputs must sit at PSUM
partition 0, so DR classes use a [64,128] slot (int|sq) and mixed classes
a [128,64] slot (sq at partitions 0-63, intersect at 64-127). ACT copies
each finished slot to a bf16 stats tile; the diagonals are extracted on
the host, which also does bincount(labels) and the final dice ratio,
like the previous version.
"""
import numpy as np
import ml_dtypes
import concourse.bacc as bacc
import concourse.mybir as mybir
import concourse.tile as tile
from concourse.bass_utils import run_bass_kernel_spmd

N_CORES = 8
B, C, X, Y, Z = 2, 33, 96, 96, 96
XS = X // N_CORES
VOX = B * XS * Y * Z          # 221184
P = 128
QC = VOX // 256               # 864 q-cols
FB = 2 * QC                   # 1728 flat cols
W = 64                        # matmul window (psum diag width)
SMOOTH = 1e-5

# classes whose intersect runs DoubleRow with an fp8 mask (tunable split:
# fp8 masks cost ~2x on DVE but their matmuls cost ~4x less on PE)
DR_CLASSES = frozenset({2, 5, 8, 11, 14, 17, 20, 23, 26, 28, 30, 31, 32})

_cached = {}


def _windows(total, w):
    out = []
    o = 0
    while o < total:
        out.append((o, min(w, total - o)))
        o += w
    return out


def _build(dr_classes=DR_CLASSES):
    n_dr = len(dr_classes)
    n_mx = C - n_dr
    nc = bacc.Bacc("TRN2", target_bir_lowering=False, debug=False,
                   num_devices=N_CORES)
    f8 = mybir.dt.float8e4
    bf = mybir.dt.bfloat16
    f32 = mybir.dt.float32
    x_in = nc.dram_tensor("x", [C, P, 2, QC], f8, kind="ExternalInput")
    lab_in = nc.dram_tensor("lab", [P, FB], bf, kind="ExternalInput")
    st_mx_out = nc.dram_tensor("st_mx", [P, W * n_mx], bf,
                               kind="ExternalOutput")
    st_dr_out = nc.dram_tensor("st_dr", [W, 2 * W * n_dr], bf,
                               kind="ExternalOutput")

    # psum slot assignment: bank = order-index % 8, cursor per bank
    cursors = [0] * 8
    slot_off = {}
    for i in range(C):
        bank = i % 8
        size = 512 if i in dr_classes else 256
        byte_off = bank * 2048 + cursors[bank]
        assert cursors[bank] + size <= 2048
        cursors[bank] += size
        slot_off[i] = byte_off // 4       # f32 column offset

    with tile.TileContext(nc) as tc:
        with (
            tc.tile_pool(name="xp", bufs=1) as xp,
            tc.tile_pool(name="labp", bufs=1) as labp,
            tc.tile_pool(name="maskp8", bufs=4) as maskp8,
            tc.tile_pool(name="maskpb", bufs=5) as maskpb,
            tc.tile_pool(name="statp", bufs=1) as statp,
            tc.tile_pool(name="psum", bufs=1, space="PSUM") as psp,
        ):
            lab = labp.tile([P, FB], bf)
            xt = xp.tile([P, C, 2, QC], f8)
            # x DMA groups: one class, then labels, then the rest
            nc.sync.dma_start(xt[:, 0:1], x_in[0:1])
            nc.sync.dma_start(lab[:], lab_in[:, :])
            for c0, c1 in [(1, 2), (2, 3), (3, 5), (5, 8), (8, 13), (13, 18),
                           (18, 23), (23, 28), (28, 33)]:
                nc.sync.dma_start(xt[:, c0:c1], x_in[c0:c1])

            ps = psp.tile([P, 4096], f32)
            st_mx = statp.tile([P, W * n_mx], bf)
            st_dr = statp.tile([W, 2 * W * n_dr], bf)

            wl = _windows(QC, W)
            mi = di = 0
            mx_shipped = dr_shipped = 0
            masks = {}

            def emit_sq_and_mask(c):
                # sumsq (DR) needs no mask; its start=True marks the 2KB zero
                # region for partitions 0-63 of the slot's bank.
                sc = slot_off[c]
                xc = xt[:, c]
                off = W if c in dr_classes else 0
                for j, (o, m) in enumerate(wl):
                    nc.tensor.matmul(
                        ps[0:m, sc + off:sc + off + m],
                        xc[:, :, o:o + m],
                        xc[:, :, o:o + m],
                        start=(j == 0),
                        stop=(c not in dr_classes and j == len(wl) - 1),
                        perf_mode=mybir.MatmulPerfMode.DoubleRow,
                        skip_group_check=True)
                if c in dr_classes:
                    mk = maskp8.tile([P, 2, QC], f8)
                else:
                    mk = maskpb.tile([P, 2, QC], bf)
                nc.vector.tensor_scalar(mk[:], lab[:], float(c), None,
                                        mybir.AluOpType.is_equal)
                masks[c] = mk

            def emit_int_and_copy(c):
                # runs one class later, so the mask is long since built and
                # PE's wait queue never blocks on DVE
                nonlocal mi, di
                sc = slot_off[c]
                xc = xt[:, c]
                mk = masks.pop(c)
                if c in dr_classes:
                    # intersect (DR) cols sc..sc+64, partitions 0-63;
                    # start=False: the sumsq start already marked this region
                    for j, (o, m) in enumerate(wl):
                        nc.tensor.matmul(
                            ps[0:m, sc:sc + m],
                            mk[:, :, o:o + m],
                            xc[:, :, o:o + m],
                            start=False, stop=(j == len(wl) - 1),
                            perf_mode=mybir.MatmulPerfMode.DoubleRow,
                            skip_group_check=True)
                    nc.scalar.copy(st_dr[:, di * 2 * W:(di + 1) * 2 * W],
                                   ps[0:W, sc:sc + 2 * W])
                    di += 1
                else:
                    # intersect (plain, bf16 mask stationary) at partitions
                    # 64-127 (own zero-region marking: partition-range aware)
                    k = 0
                    nwin = 2 * len(wl)
                    for t in range(2):
                        for (o, m) in wl:
                            nc.tensor.matmul(
                                ps[W:W + m, sc:sc + m],
                                mk[:, t, o:o + m],
                                xc[:, t, o:o + m],
                                start=(k == 0), stop=(k == nwin - 1),
                                skip_group_check=True)
                            k += 1
                    nc.scalar.copy(st_mx[:, mi * W:(mi + 1) * W],
                                   ps[:, sc:sc + W])
                    mi += 1

            def ship():
                nonlocal mx_shipped, dr_shipped
                if mi > mx_shipped:
                    nc.sync.dma_start(st_mx_out[:, mx_shipped * W:mi * W],
                                      st_mx[:, mx_shipped * W:mi * W])
                    mx_shipped = mi
                if di > dr_shipped:
                    nc.sync.dma_start(
                        st_dr_out[:, dr_shipped * 2 * W:di * 2 * W],
                        st_dr[:, dr_shipped * 2 * W:di * 2 * W])
                    dr_shipped = di

            for c in range(C):
                emit_sq_and_mask(c)
                if c >= 1:
                    emit_int_and_copy(c - 1)
                if c in (12, 20, 26, 29, 31):
                    ship()
            emit_int_and_copy(C - 1)
            ship()
    nc.compile()
    return nc


def _get_nc():
    if "nc" not in _cached:
        _cached["nc"] = _build()
    return _cached["nc"]


def kernel(outputs, label):
    nc = _get_nc()
    outputs = np.asarray(outputs)
    lab_np = np.asarray(label)
    f8 = ml_dtypes.bfloat16  # placeholder; real fp8 below
    fp8 = ml_dtypes.float8_e4m3
    bf16 = ml_dtypes.bfloat16
    in_maps = []
    for k in range(N_CORES):
        xs = outputs[:, :, k * XS:(k + 1) * XS]          # [B, C, XS, Y, Z]
        xs = np.ascontiguousarray(xs.transpose(1, 0, 2, 3, 4)).reshape(C, VOX)
        # v = q*256 + t*128 + p  ->  [C, 864, 2, 128] -> [C, 128, 2, 864]
        xd = np.ascontiguousarray(
            xs.reshape(C, QC, 2, P).transpose(0, 3, 2, 1)).astype(fp8)
        ls = lab_np[:, k * XS:(k + 1) * XS].reshape(VOX)
        ld = np.ascontiguousarray(
            ls.reshape(QC, 2, P).transpose(2, 1, 0)).reshape(P, FB)
        in_maps.append({"x": xd, "lab": ld.astype(bf16)})
    res = run_bass_kernel_spmd(nc, in_maps, core_ids=list(range(N_CORES)))

    dr_sorted = sorted(DR_CLASSES)
    mx_sorted = [c for c in range(C) if c not in DR_CLASSES]
    intersect = np.zeros(C, np.float64)
    sumsq = np.zeros(C, np.float64)
    idx = np.arange(W)
    for r in res.results:
        smx = r["st_mx"].astype(np.float64)   # [128, 64*n_mx]
        sdr = r["st_dr"].astype(np.float64)   # [64, 128*n_dr]
        for mi, c in enumerate(mx_sorted):
            blk = smx[:, mi * W:(mi + 1) * W]
            sumsq[c] += blk[idx, idx].sum()
            intersect[c] += blk[W + idx, idx].sum()
        for di, c in enumerate(dr_sorted):
            blk = sdr[:, di * 2 * W:(di + 1) * 2 * W]
            intersect[c] += blk[idx, idx].sum()
            sumsq[c] += blk[idx, W + idx].sum()
    labels_sum = np.bincount(
        lab_np.reshape(-1).astype(np.int64), minlength=C).astype(np.float64)
    dice = (2.0 * intersect + SMOOTH) / (sumsq + labels_sum + SMOOTH)
    return np.float32(np.mean(1.0 - dice))


# revision 16
# speedup vs baseline: 1.0260x; 1.0139x over previous
"""DiceLoss kernel v2: fp8 x + DoubleRow matmuls + host-side diag extraction.

Per core (1/8 of the X axis): x is shipped as fp8e4 in a [33, 128, 2, 864]
layout (voxel v = q*256 + t*128 + p), labels as bf16 [128, 1728]
(flat col j = t*864 + q). Per class c:
  sumsq  = diag of DoubleRow x^T x (fp8, 0.5 cyc/row on PE)
  intersect = diag of mask^T x; for DR_CLASSES the mask is fp8 and the
    matmul runs DoubleRow; otherwise the mask is bf16 (DVE 4x build) and
    the matmul streams x fp8 plain (1 cyc/row).
Each class accumulates its diagonals into a 256B/512B PSUM slot
(bank-strided, reuse gap 8 classes); DoubleRow outputs must sit at PSUM
partition 0, so DR classes use a [64,128] slot (int|sq) and mixed classes
a [128,64] slot (sq at partitions 0-63, intersect at 64-127). ACT copies
each finished slot to a bf16 stats tile; the diagonals are extracted on
the host, which also does bincount(labels) and the final dice ratio,
like the previous version.
"""
import numpy as np
import ml_dtypes
import concourse.bacc as bacc
import concourse.mybir as mybir
import concourse.tile as tile
from concourse.bass_utils import run_bass_kernel_spmd

N_CORES = 8
B, C, X, Y, Z = 2, 33, 96, 96, 96
XS = X // N_CORES
VOX = B * XS * Y * Z          # 221184
P = 128
QC = VOX // 256               # 864 q-cols
FB = 2 * QC                   # 1728 flat cols
W = 64                        # matmul window (psum diag width)
SMOOTH = 1e-5

# classes whose intersect runs DoubleRow with an fp8 mask (tunable split:
# fp8 masks cost ~2x on DVE but their matmuls cost ~4x less on PE)
DR_CLASSES = frozenset({2, 4, 5, 6, 8, 11, 14, 17, 20, 23, 26, 28, 29, 30,
                        31, 32})

_cached = {}


def _windows(total, w):
    out = []
    o = 0
    while o < total:
        out.append((o, min(w, total - o)))
        o += w
    return out


def _build(dr_classes=DR_CLASSES):
    n_dr = len(dr_classes)
    n_mx = C - n_dr
    nc = bacc.Bacc("TRN2", target_bir_lowering=False, debug=False,
                   num_devices=N_CORES)
    f8 = mybir.dt.float8e4
    bf = mybir.dt.bfloat16
    f32 = mybir.dt.float32
    x_in = nc.dram_tensor("x", [C, P, 2, QC], f8, kind="ExternalInput")
    lab_in = nc.dram_tensor("lab", [P, FB], bf, kind="ExternalInput")
    st_mx_out = nc.dram_tensor("st_mx", [P, W * n_mx], bf,
                               kind="ExternalOutput")
    st_dr_out = nc.dram_tensor("st_dr", [W, 2 * W * n_dr], bf,
                               kind="ExternalOutput")

    # psum slot assignment: bank = order-index % 8, cursor per bank
    cursors = [0] * 8
    slot_off = {}
    for i in range(C):
        bank = i % 8
        size = 512 if i in dr_classes else 256
        byte_off = bank * 2048 + cursors[bank]
        assert cursors[bank] + size <= 2048
        cursors[bank] += size
        slot_off[i] = byte_off // 4       # f32 column offset

    with tile.TileContext(nc) as tc:
        with (
            tc.tile_pool(name="xp", bufs=1) as xp,
            tc.tile_pool(name="labp", bufs=1) as labp,
            tc.tile_pool(name="maskp8", bufs=4) as maskp8,
            tc.tile_pool(name="maskpb", bufs=5) as maskpb,
            tc.tile_pool(name="statp", bufs=1) as statp,
            tc.tile_pool(name="psum", bufs=1, space="PSUM") as psp,
        ):
            lab = labp.tile([P, FB], bf)
            xt = xp.tile([P, C, 2, QC], f8)
            # x DMA groups: one class, then labels, then the rest
            nc.sync.dma_start(xt[:, 0:1], x_in[0:1])
            nc.sync.dma_start(lab[:], lab_in[:, :])
            for c0, c1 in [(1, 2), (2, 3), (3, 5), (5, 8), (8, 13), (13, 18),
                           (18, 23), (23, 28), (28, 33)]:
                nc.sync.dma_start(xt[:, c0:c1], x_in[c0:c1])

            ps = psp.tile([P, 4096], f32)
            st_mx = statp.tile([P, W * n_mx], bf)
            st_dr = statp.tile([W, 2 * W * n_dr], bf)

            wl = _windows(QC, W)
            mi = di = 0
            mx_shipped = dr_shipped = 0
            masks = {}

            def emit_sq_and_mask(c):
                # sumsq (DR) needs no mask; its start=True marks the 2KB zero
                # region for partitions 0-63 of the slot's bank.
                sc = slot_off[c]
                xc = xt[:, c]
                off = W if c in dr_classes else 0
                for j, (o, m) in enumerate(wl):
                    nc.tensor.matmul(
                        ps[0:m, sc + off:sc + off + m],
                        xc[:, :, o:o + m],
                        xc[:, :, o:o + m],
                        start=(j == 0),
                        stop=(c not in dr_classes and j == len(wl) - 1),
                        perf_mode=mybir.MatmulPerfMode.DoubleRow,
                        skip_group_check=True)
                if c in dr_classes:
                    mk = maskp8.tile([P, 2, QC], f8)
                else:
                    mk = maskpb.tile([P, 2, QC], bf)
                nc.vector.tensor_scalar(mk[:], lab[:], float(c), None,
                                        mybir.AluOpType.is_equal)
                masks[c] = mk

            def emit_int_and_copy(c):
                # runs one class later, so the mask is long since built and
                # PE's wait queue never blocks on DVE
                nonlocal mi, di
                sc = slot_off[c]
                xc = xt[:, c]
                mk = masks.pop(c)
                if c in dr_classes:
                    # intersect (DR) cols sc..sc+64, partitions 0-63;
                    # start=False: the sumsq start already marked this region
                    for j, (o, m) in enumerate(wl):
                        nc.tensor.matmul(
                            ps[0:m, sc:sc + m],
                            mk[:, :, o:o + m],
                            xc[:, :, o:o + m],
                            start=False, stop=(j == len(wl) - 1),
                            perf_mode=mybir.MatmulPerfMode.DoubleRow,
                            skip_group_check=True)
                    nc.scalar.copy(st_dr[:, di * 2 * W:(di + 1) * 2 * W],
                                   ps[0:W, sc:sc + 2 * W])
                    di += 1
                else:
                    # intersect (plain, bf16 mask stationary) at partitions
                    # 64-127 (own zero-region marking: partition-range aware)
                    k = 0
                    nwin = 2 * len(wl)
                    for t in range(2):
                        for (o, m) in wl:
                            nc.tensor.matmul(
                                ps[W:W + m, sc:sc + m],
                                mk[:, t, o:o + m],
                                xc[:, t, o:o + m],
                                start=(k == 0), stop=(k == nwin - 1),
                                skip_group_check=True)
                            k += 1
                    nc.scalar.copy(st_mx[:, mi * W:(mi + 1) * W],
                                   ps[:, sc:sc + W])
                    mi += 1

            def ship():
                nonlocal mx_shipped, dr_shipped
                if mi > mx_shipped:
                    nc.sync.dma_start(st_mx_out[:, mx_shipped * W:mi * W],
                                      st_mx[:, mx_shipped * W:mi * W])
                    mx_shipped = mi
                if di > dr_shipped:
                    nc.sync.dma_start(
                        st_dr_out[:, dr_shipped * 2 * W:di * 2 * W],
                        st_dr[:, dr_shipped * 2 * W:di * 2 * W])
                    dr_shipped = di

            for c in range(C):
                emit_sq_and_mask(c)
                if c >= 1:
                    emit_int_and_copy(c - 1)
                if c in (12, 20, 26, 29, 31):
                    ship()
            emit_int_and_copy(C - 1)
            ship()
    nc.compile()
    return nc


def _get_nc():
    if "nc" not in _cached:
        _cached["nc"] = _build()
    return _cached["nc"]


def kernel(outputs, label):
    nc = _get_nc()
    outputs = np.asarray(outputs)
    lab_np = np.asarray(label)
    fp8 = ml_dtypes.float8_e4m3
    bf16 = ml_dtypes.bfloat16
    in_maps = []
    for k in range(N_CORES):
        xs = outputs[:, :, k * XS:(k + 1) * XS]          # [B, C, XS, Y, Z]
        xs = np.ascontiguousarray(xs.transpose(1, 0, 2, 3, 4)).reshape(C, VOX)
        # v = q*256 + t*128 + p  ->  [C, 864, 2, 128] -> [C, 128, 2, 864]
        xd = np.ascontiguousarray(
            xs.reshape(C, QC, 2, P).transpose(0, 3, 2, 1)).astype(fp8)
        ls = lab_np[:, k * XS:(k + 1) * XS].reshape(VOX)
        ld = np.ascontiguousarray(
            ls.reshape(QC, 2, P).transpose(2, 1, 0)).reshape(P, FB)
        in_maps.append({"x": xd, "lab": ld.astype(bf16)})
    res = run_bass_kernel_spmd(nc, in_maps, core_ids=list(range(N_CORES)))

    dr_sorted = sorted(DR_CLASSES)
    mx_sorted = [c for c in range(C) if c not in DR_CLASSES]
    intersect = np.zeros(C, np.float64)
    sumsq = np.zeros(C, np.float64)
    idx = np.arange(W)
    for r in res.results:
        smx = r["st_mx"].astype(np.float64)   # [128, 64*n_mx]
        sdr = r["st_dr"].astype(np.float64)   # [64, 128*n_dr]
        for mi, c in enumerate(mx_sorted):
            blk = smx[:, mi * W:(mi + 1) * W]
            sumsq[c] += blk[idx, idx].sum()
            intersect[c] += blk[W + idx, idx].sum()
        for di, c in enumerate(dr_sorted):
            blk = sdr[:, di * 2 * W:(di + 1) * 2 * W]
            intersect[c] += blk[idx, idx].sum()
            sumsq[c] += blk[idx, W + idx].sum()
    labels_sum = np.bincount(
        lab_np.reshape(-1).astype(np.int64), minlength=C).astype(np.float64)
    dice = (2.0 * intersect + SMOOTH) / (sumsq + labels_sum + SMOOTH)
    return np.float32(np.mean(1.0 - dice))


# revision 17
# speedup vs baseline: 1.0388x; 1.0124x over previous
"""DiceLoss kernel v2: fp8 x + DoubleRow matmuls + host-side diag extraction.

Per core (1/8 of the X axis): x is shipped as fp8e4 in a [33, 128, 2, 864]
layout (voxel v = q*256 + t*128 + p), labels as bf16 [128, 1728]
(flat col j = t*864 + q). Per class c:
  sumsq  = diag of DoubleRow x^T x (fp8, 0.5 cyc/row on PE)
  intersect = diag of mask^T x; for DR_CLASSES the mask is fp8 and the
    matmul runs DoubleRow; otherwise the mask is bf16 (DVE 4x build) and
    the matmul streams x fp8 plain (1 cyc/row).
Each class accumulates its diagonals into a 256B/512B PSUM slot
(bank-strided, reuse gap 8 classes); DoubleRow outputs must sit at PSUM
partition 0, so DR classes use a [64,128] slot (int|sq) and mixed classes
a [128,64] slot (sq at partitions 0-63, intersect at 64-127). ACT copies
each finished slot to a bf16 stats tile; the diagonals are extracted on
the host, which also does bincount(labels) and the final dice ratio,
like the previous version.
"""
import numpy as np
import ml_dtypes
import concourse.bacc as bacc
import concourse.mybir as mybir
import concourse.tile as tile
from concourse.bass_utils import run_bass_kernel_spmd

N_CORES = 8
B, C, X, Y, Z = 2, 33, 96, 96, 96
XS = X // N_CORES
VOX = B * XS * Y * Z          # 221184
P = 128
QC = VOX // 256               # 864 q-cols
FB = 2 * QC                   # 1728 flat cols
W = 64                        # matmul window (psum diag width)
SMOOTH = 1e-5

# classes whose intersect runs DoubleRow with an fp8 mask (tunable split:
# fp8 masks cost ~2x on DVE but their matmuls cost ~4x less on PE)
DR_CLASSES = frozenset({2, 5, 6, 8, 11, 14, 17, 20, 23, 26, 28, 29, 30,
                        31, 32})

_cached = {}


def _windows(total, w):
    out = []
    o = 0
    while o < total:
        out.append((o, min(w, total - o)))
        o += w
    return out


def _build(dr_classes=DR_CLASSES):
    n_dr = len(dr_classes)
    n_mx = C - n_dr
    nc = bacc.Bacc("TRN2", target_bir_lowering=False, debug=False,
                   num_devices=N_CORES)
    f8 = mybir.dt.float8e4
    bf = mybir.dt.bfloat16
    f32 = mybir.dt.float32
    x_in = nc.dram_tensor("x", [C, P, 2, QC], f8, kind="ExternalInput")
    lab_in = nc.dram_tensor("lab", [P, FB], bf, kind="ExternalInput")
    st_mx_out = nc.dram_tensor("st_mx", [P, W * n_mx], bf,
                               kind="ExternalOutput")
    st_dr_out = nc.dram_tensor("st_dr", [W, 2 * W * n_dr], bf,
                               kind="ExternalOutput")

    # psum slot assignment: bank = order-index % 8, cursor per bank
    cursors = [0] * 8
    slot_off = {}
    for i in range(C):
        bank = i % 8
        size = 512 if i in dr_classes else 256
        byte_off = bank * 2048 + cursors[bank]
        assert cursors[bank] + size <= 2048
        cursors[bank] += size
        slot_off[i] = byte_off // 4       # f32 column offset

    with tile.TileContext(nc) as tc:
        with (
            tc.tile_pool(name="xp", bufs=1) as xp,
            tc.tile_pool(name="labp", bufs=1) as labp,
            tc.tile_pool(name="maskp8", bufs=4) as maskp8,
            tc.tile_pool(name="maskpb", bufs=5) as maskpb,
            tc.tile_pool(name="statp", bufs=1) as statp,
            tc.tile_pool(name="psum", bufs=1, space="PSUM") as psp,
        ):
            lab = labp.tile([P, FB], bf)
            xt = xp.tile([P, C, 2, QC], f8)
            # x DMA groups: one class, then labels, then the rest
            nc.sync.dma_start(xt[:, 0:1], x_in[0:1])
            nc.sync.dma_start(lab[:], lab_in[:, :])
            for c0, c1 in [(1, 2), (2, 3), (3, 5), (5, 8), (8, 13), (13, 18),
                           (18, 23), (23, 28), (28, 33)]:
                nc.sync.dma_start(xt[:, c0:c1], x_in[c0:c1])

            ps = psp.tile([P, 4096], f32)
            st_mx = statp.tile([P, W * n_mx], bf)
            st_dr = statp.tile([W, 2 * W * n_dr], bf)

            wl = _windows(QC, W)
            mi = di = 0
            mx_shipped = dr_shipped = 0
            masks = {}

            def emit_sq_and_mask(c):
                # sumsq (DR) needs no mask; its start=True marks the 2KB zero
                # region for partitions 0-63 of the slot's bank.
                sc = slot_off[c]
                xc = xt[:, c]
                off = W if c in dr_classes else 0
                for j, (o, m) in enumerate(wl):
                    nc.tensor.matmul(
                        ps[0:m, sc + off:sc + off + m],
                        xc[:, :, o:o + m],
                        xc[:, :, o:o + m],
                        start=(j == 0),
                        stop=(c not in dr_classes and j == len(wl) - 1),
                        perf_mode=mybir.MatmulPerfMode.DoubleRow,
                        skip_group_check=True)
                if c in dr_classes:
                    mk = maskp8.tile([P, 2, QC], f8)
                else:
                    mk = maskpb.tile([P, 2, QC], bf)
                nc.vector.tensor_scalar(mk[:], lab[:], float(c), None,
                                        mybir.AluOpType.is_equal)
                masks[c] = mk

            def emit_int_and_copy(c):
                # runs one class later, so the mask is long since built and
                # PE's wait queue never blocks on DVE
                nonlocal mi, di
                sc = slot_off[c]
                xc = xt[:, c]
                mk = masks.pop(c)
                if c in dr_classes:
                    # intersect (DR) cols sc..sc+64, partitions 0-63;
                    # start=False: the sumsq start already marked this region
                    for j, (o, m) in enumerate(wl):
                        nc.tensor.matmul(
                            ps[0:m, sc:sc + m],
                            mk[:, :, o:o + m],
                            xc[:, :, o:o + m],
                            start=False, stop=(j == len(wl) - 1),
                            perf_mode=mybir.MatmulPerfMode.DoubleRow,
                            skip_group_check=True)
                    nc.scalar.copy(st_dr[:, di * 2 * W:(di + 1) * 2 * W],
                                   ps[0:W, sc:sc + 2 * W])
                    di += 1
                else:
                    # intersect (plain, bf16 mask stationary) at partitions
                    # 64-127 (own zero-region marking: partition-range aware)
                    k = 0
                    nwin = 2 * len(wl)
                    for t in range(2):
                        for (o, m) in wl:
                            nc.tensor.matmul(
                                ps[W:W + m, sc:sc + m],
                                mk[:, t, o:o + m],
                                xc[:, t, o:o + m],
                                start=(k == 0), stop=(k == nwin - 1),
                                skip_group_check=True)
                            k += 1
                    nc.scalar.copy(st_mx[:, mi * W:(mi + 1) * W],
                                   ps[:, sc:sc + W])
                    mi += 1

            def ship():
                nonlocal mx_shipped, dr_shipped
                if mi > mx_shipped:
                    nc.sync.dma_start(st_mx_out[:, mx_shipped * W:mi * W],
                                      st_mx[:, mx_shipped * W:mi * W])
                    mx_shipped = mi
                if di > dr_shipped:
                    nc.sync.dma_start(
                        st_dr_out[:, dr_shipped * 2 * W:di * 2 * W],
                        st_dr[:, dr_shipped * 2 * W:di * 2 * W])
                    dr_shipped = di

            for c in range(C):
                emit_sq_and_mask(c)
                if c >= 1:
                    emit_int_and_copy(c - 1)
                if c in (12, 20, 26, 29, 31):
                    ship()
            emit_int_and_copy(C - 1)
            ship()
    nc.compile()
    return nc


def _get_nc():
    if "nc" not in _cached:
        _cached["nc"] = _build()
    return _cached["nc"]


def kernel(outputs, label):
    nc = _get_nc()
    outputs = np.asarray(outputs)
    lab_np = np.asarray(label)
    fp8 = ml_dtypes.float8_e4m3
    bf16 = ml_dtypes.bfloat16
    in_maps = []
    for k in range(N_CORES):
        xs = outputs[:, :, k * XS:(k + 1) * XS]          # [B, C, XS, Y, Z]
        xs = np.ascontiguousarray(xs.transpose(1, 0, 2, 3, 4)).reshape(C, VOX)
        # v = q*256 + t*128 + p  ->  [C, 864, 2, 128] -> [C, 128, 2, 864]
        xd = np.ascontiguousarray(
            xs.reshape(C, QC, 2, P).transpose(0, 3, 2, 1)).astype(fp8)
        ls = lab_np[:, k * XS:(k + 1) * XS].reshape(VOX)
        ld = np.ascontiguousarray(
            ls.reshape(QC, 2, P).transpose(2, 1, 0)).reshape(P, FB)
        in_maps.append({"x": xd, "lab": ld.astype(bf16)})
    res = run_bass_kernel_spmd(nc, in_maps, core_ids=list(range(N_CORES)))

    dr_sorted = sorted(DR_CLASSES)
    mx_sorted = [c for c in range(C) if c not in DR_CLASSES]
    intersect = np.zeros(C, np.float64)
    sumsq = np.zeros(C, np.float64)
    idx = np.arange(W)
    for r in res.results:
        smx = r["st_mx"].astype(np.float64)   # [128, 64*n_mx]
        sdr = r["st_dr"].astype(np.float64)   # [64, 128*n_dr]
        for mi, c in enumerate(mx_sorted):
            blk = smx[:, mi * W:(mi + 1) * W]
            sumsq[c] += blk[idx, idx].sum()
            intersect[c] += blk[W + idx, idx].sum()
        for di, c in enumerate(dr_sorted):
            blk = sdr[:, di * 2 * W:(di + 1) * 2 * W]
            intersect[c] += blk[idx, idx].sum()
            sumsq[c] += blk[idx, W + idx].sum()
    labels_sum = np.bincount(
        lab_np.reshape(-1).astype(np.int64), minlength=C).astype(np.float64)
    dice = (2.0 * intersect + SMOOTH) / (sumsq + labels_sum + SMOOTH)
    return np.float32(np.mean(1.0 - dice))
